# revision 24
# baseline (speedup 1.0000x reference)
"""Multi-head attention with bias, distributed over 8 trn2 NeuronCores.

Reference computation (per batch b):
    q = (x @ Wq.T) * depth**-0.5 ; k = y @ Wk.T ; v = y @ Wv.T     (per-head split)
    out = softmax(q @ k.T + bias) @ v @ Wo.T

Sharding: 8 cores = 4 batches x 2 head-halves (tensor parallel over heads).
Core c handles batch b = c//2 and heads (c%2)*8 .. +8.  Wq/Wk/Wv are
column-split, Wo row-split; the two partial outputs per batch are summed on
the host (no device collective).

Device-side layout (feature dim on partitions):
    qT/kT = W.T-projected activations [d_out=512, 2048]; v natural [kk, h, d].
    logitsT[kk, q] per head via row-tiled K=64 matmul pairs (2 heads share
    the 128-partition d-tile; tile_position rows 0-63 / 64-127 concurrent)
    expw = exp(logitsT) * exp(bias).T      (exp(bias) precomputed on host,
                                            streamed per (pair, q-chunk))
    attnT_h(+denom row) = [v_h | ones].T @ expw  (K=128, denom rides row 64)
    normalize via DVE reciprocal + DMA partition-broadcast from DRAM
    outT_partial = Wo_half.T-proj of normalized attnT (summed on host).

Scheduling: attn matmuls LAG two iterations behind their logits pair so the
in-order PE queue never blocks on the ACT->DVE chain; the normalization
epilogue is split into three stages deferred across following iterations so
its DMA round-trips never head-block the DVE queue; projection matmuls are
chopped into per-MM filler ops drained into PE slack with (deadline,
earliest) emission gates.  Inputs arrive as one large striped DMA per
tensor in critical-path order.
Host does: transposes, bf16 casts, exp(bias) pre-tiling, scale fold into Wq.
"""

import numpy as np
import ml_dtypes
from collections import deque
from contextlib import ExitStack

import concourse.bass as bass
import concourse.mybir as mybir
import concourse.tile as tile
from concourse import bacc
from concourse.bass_utils import run_bass_kernel_spmd

# full-problem dims (hardcoded per spec)
B, S, D, H = 4, 2048, 1024, 16
DEPTH = D // H            # 64
P = 128
NCORES = 8

DH = D // 2               # 512 head dims per core (8 heads)
NPAIR = 4                 # head pairs per core
NCH = 4                   # q chunks of 512
KT = S // P               # 16 kk tiles
NU = D // P               # 8 d_in tiles
CW = 512                  # q chunk width
LAG = 2                   # attn matmul lag (iterations) behind logits

BF = mybir.dt.bfloat16
F32 = mybir.dt.float32
EXP = mybir.ActivationFunctionType.Exp

TRACE = False
last_exec_time_ns = None
last_results = None


def _attn_body(ctx, tc, io):
    nc = tc.nc
    xT, yT, wqT, wkT, wvT, woT, ebt, outT = (
        io[k] for k in ("xT", "yT", "wqT", "wkT", "wvT", "woT", "ebt", "outT"))

    # ---------------- persistent pools ----------------
    qpool = ctx.enter_context(tc.tile_pool(name="qpool", bufs=NPAIR))
    kpool = ctx.enter_context(tc.tile_pool(name="kpool", bufs=NPAIR))
    vpool = ctx.enter_context(tc.tile_pool(name="vpool", bufs=KT))
    anpool = ctx.enter_context(tc.tile_pool(name="anpool", bufs=NPAIR))
    ebpool = ctx.enter_context(tc.tile_pool(name="ebpool", bufs=16))
    epool = ctx.enter_context(tc.tile_pool(name="epool", bufs=8))
    smpool = ctx.enter_context(tc.tile_pool(name="smpool", bufs=4))
    plp = ctx.enter_context(tc.tile_pool(name="plp", bufs=2, space="PSUM"))
    pap = ctx.enter_context(tc.tile_pool(name="pap", bufs=2, space="PSUM"))
    pop = ctx.enter_context(tc.tile_pool(name="pop", bufs=2, space="PSUM"))
    dpool = ctx.enter_context(tc.tile_pool(name="dpool", bufs=2, space="DRAM"))
    wopool = ctx.enter_context(tc.tile_pool(name="wopool", bufs=NPAIR))
    opool = ctx.enter_context(tc.tile_pool(name="opool", bufs=2))

    qT_sb = [qpool.tile([P, S], BF, tag="qT", name=f"qT{p}", bufs=NPAIR)
             for p in range(NPAIR)]
    kT_sb = [kpool.tile([P, S], BF, tag="kT", name=f"kT{p}", bufs=NPAIR)
             for p in range(NPAIR)]
    v_sb = [vpool.tile([P, 2 * NPAIR, 66], BF, tag="v66", name=f"v{c}",
                       bufs=KT) for c in range(KT)]
    an_sb = [anpool.tile([P, S], BF, tag="an", name=f"an{p}", bufs=NPAIR)
             for p in range(NPAIR)]
    wo_sb = [wopool.tile([P, D], BF, tag="wo", name=f"wo{p}", bufs=NPAIR)
             for p in range(NPAIR)]

    def eb_tile(p, ch, c):
        return ebpool.tile([P, CW], BF, tag="eb", name=f"eb{p}_{ch}_{c}",
                           bufs=16)

    def dma_eb_slab(ch, tiles, c0=0, c1=KT):
        base = ch * S
        for c in range(c0, c1):
            nc.gpsimd.dma_start(out=tiles[c],
                                in_=ebt[base + c * P:base + (c + 1) * P, :])

    # ---------------- input loads + projection helpers ----------------
    with tc.tile_pool(name="ypool", bufs=1) as ypool, \
         tc.tile_pool(name="xpool", bufs=1) as xpool, \
         tc.tile_pool(name="wkpool", bufs=1) as wkpool, \
         tc.tile_pool(name="wqpool", bufs=1) as wqpool, \
         tc.tile_pool(name="wvpool", bufs=1) as wvpool:
        wk_sb = [wkpool.tile([P, NU, P], BF, tag="wk", name=f"wk{p}",
                             bufs=NPAIR) for p in range(NPAIR)]
        wq_sb = [wqpool.tile([P, NU, P], BF, tag="wq", name=f"wq{p}",
                             bufs=NPAIR) for p in range(NPAIR)]
        wv_sb = wvpool.tile([P, NU, DH], BF, tag="wv", name="wv")
        y_sb = ypool.tile([P, NU, S], BF, tag="yT", name="y")
        x_sb = xpool.tile([P, NU, S], BF, tag="xT", name="x")

        def dma_pair_w(dst, src, p):
            nc.sync.dma_start(
                out=dst[p],
                in_=src[p * D:(p + 1) * D, :].rearrange("(u p) j -> p u j",
                                                        p=P))

        def dma_slab(dst, src, ch):
            # 256KB units of 2 row-blocks: near-sequential DRAM read order
            for u in range(0, NU, 2):
                nc.sync.dma_start(
                    out=dst[:, u:u + 2, ch * CW:(ch + 1) * CW],
                    in_=src[ch * D + u * P:ch * D + (u + 2) * P, :].rearrange(
                        "(u p) j -> p u j", p=P))

        # pre-tiled transfers, strictly in first-use order
        dma_pair_w(wk_sb, wkT, 0)
        dma_slab(y_sb, yT, 0)
        dma_pair_w(wq_sb, wqT, 0)
        dma_slab(x_sb, xT, 0)
        for u in range(0, NU, 2):
            nc.sync.dma_start(
                out=wv_sb[:, u:u + 2, :],
                in_=wvT[u * P:(u + 2) * P, :].rearrange("(u p) j -> p u j",
                                                        p=P))
        dma_slab(y_sb, yT, 1)
        dma_slab(y_sb, yT, 2)
        dma_pair_w(wk_sb, wkT, 1)
        dma_slab(y_sb, yT, 3)
        dma_slab(x_sb, xT, 1)
        dma_pair_w(wq_sb, wqT, 1)
        dma_pair_w(wk_sb, wkT, 2)
        dma_pair_w(wk_sb, wkT, 3)
        dma_pair_w(wq_sb, wqT, 2)
        dma_pair_w(wq_sb, wqT, 3)
        dma_slab(x_sb, xT, 2)
        dma_slab(x_sb, xT, 3)
        # eb slab 0 on the gpsimd queue runs concurrently from t=0
        eb0 = [eb_tile(0, 0, c) for c in range(KT)]
        dma_eb_slab(0, eb0)

        # warm-up heartbeats chained to arriving inputs keep the PE HAM
        # activity window alive through the load phase
        jnk0 = plp.tile([1, 1024], F32, tag="pl", name="jnk0", bufs=2)
        for t in (wk_sb[0], wv_sb, y_sb):
            nc.tensor.matmul(jnk0[0:1, 0:P], lhsT=t[0:1, 0, 0:1],
                             rhs=t[0:1, 0, 0:P], start=True, stop=True)

        # ---- per-MM filler ops ----
        def k_group_ops(p, kkc):
            box = {}
            def mm(u, box=box):
                if u == 0:
                    box["ps"] = pop.tile([P, CW], F32, tag="po",
                                         name=f"psk{p}_{kkc}", bufs=2)
                nc.tensor.matmul(box["ps"],
                                 lhsT=wk_sb[p][:, u, :],
                                 rhs=y_sb[:, u, kkc * CW:(kkc + 1) * CW],
                                 start=(u == 0), stop=(u == NU - 1))
            def fin(box=box):
                nc.vector.tensor_copy(
                    kT_sb[p][:, kkc * CW:(kkc + 1) * CW], box["ps"])
            return [lambda u=u, mm=mm: mm(u) for u in range(NU)] + [fin]

        def q_group_ops(p, ch):
            box = {}
            def mm(u, box=box):
                if u == 0:
                    box["ps"] = pop.tile([P, CW], F32, tag="po",
                                         name=f"psq{p}_{ch}", bufs=2)
                nc.tensor.matmul(box["ps"],
                                 lhsT=wq_sb[p][:, u, :],
                                 rhs=x_sb[:, u, ch * CW:(ch + 1) * CW],
                                 start=(u == 0), stop=(u == NU - 1))
            def fin(box=box):
                nc.vector.tensor_copy(
                    qT_sb[p][:, ch * CW:(ch + 1) * CW], box["ps"])
            return [lambda u=u, mm=mm: mm(u) for u in range(NU)] + [fin]

        def v_group_ops(c):
            box = {}
            def mm(u, box=box):
                if u == 0:
                    box["ps"] = pop.tile([P, CW], F32, tag="po",
                                         name=f"psv{c}", bufs=2)
                nc.tensor.matmul(box["ps"],
                                 lhsT=y_sb[:, u, c * P:(c + 1) * P],
                                 rhs=wv_sb[:, u, 0:DH],
                                 start=(u == 0), stop=(u == NU - 1))
            def fin(box=box):
                vt = v_sb[c]
                nc.vector.memset(vt[:, :, 64:65], 1.0)
                nc.vector.memset(vt[:, :, 65:66], 0.0)
                src = box["ps"].rearrange("p (g d) -> p g d", d=DEPTH)
                nc.vector.tensor_copy(vt[:, :, 0:DEPTH], src)
            return [lambda u=u, mm=mm: mm(u) for u in range(NU)] + [fin]

        def load_wo(p):
            nc.gpsimd.dma_start(out=wo_sb[p], in_=woT[p * P:(p + 1) * P, :])

        def out_group_ops(m, ch):
            box = {}
            def mm(p4, box=box):
                if p4 == 0:
                    box["ps"] = pop.tile([P, CW], F32, tag="po",
                                         name=f"pso{m}_{ch}", bufs=2)
                nc.tensor.matmul(box["ps"],
                                 lhsT=wo_sb[p4][:, m * P:(m + 1) * P],
                                 rhs=an_sb[p4][:, ch * CW:(ch + 1) * CW],
                                 start=(p4 == 0), stop=(p4 == NPAIR - 1))
            def fin(box=box):
                osb = opool.tile([P, CW], F32, tag="osb", name=f"o{m}_{ch}",
                                 bufs=2)
                nc.vector.tensor_copy(osb, box["ps"])
                nc.sync.dma_start(
                    out=outT[m * P:(m + 1) * P, ch * CW:(ch + 1) * CW],
                    in_=osb)
            return [lambda p4=p4, mm=mm: mm(p4) for p4 in range(NPAIR)] + [fin]

        # ---------------- attention emission ----------------
        state = {"eb": {(0, 0): eb0}}

        def norm_a(p, ch, pattn):
            """Stage A (at last attn MM): drain psum, gather denominators."""
            saus = []
            for hf in range(2):
                sau = smpool.tile([65, CW], BF, tag="sau",
                                  name=f"sa{p}_{ch}_{hf}", bufs=3)
                saus.append(sau)
                nc.vector.tensor_copy(sau, pattn[hf])
            den_t = smpool.tile([2, CW], BF, tag="dent",
                                name=f"den{p}_{ch}", bufs=1)
            for hf in range(2):
                nc.sync.dma_start(out=den_t[hf:hf + 1, :],
                                  in_=saus[hf][64:65, :])
            return saus, den_t

        def norm_b(p, ch, den_t):
            """Stage B (+1 iter): reciprocal + DRAM broadcast round-trip."""
            denf = smpool.tile([2, CW], F32, tag="denf",
                               name=f"dnf{p}_{ch}", bufs=1)
            nc.vector.tensor_copy(denf, den_t)
            recipf = smpool.tile([2, CW], F32, tag="recipf",
                                 name=f"rcf{p}_{ch}", bufs=1)
            nc.vector.reciprocal_approx_fast(recipf, denf)
            recipb = smpool.tile([2, CW], BF, tag="recipb",
                                 name=f"rcb{p}_{ch}", bufs=1)
            nc.vector.tensor_copy(recipb, recipf)
            rscr = dpool.tile([2, CW], BF, tag="rscr",
                              name=f"rs{p}_{ch}", bufs=2)
            nc.sync.dma_start(out=rscr, in_=recipb)
            bcs = []
            for hf in range(2):
                bc = smpool.tile([DEPTH, CW], BF, tag="bc",
                                 name=f"bc{p}_{ch}_{hf}", bufs=2)
                bcs.append(bc)
                nc.sync.dma_start(
                    out=bc, in_=rscr[hf:hf + 1, :].partition_broadcast(DEPTH))
            return bcs

        def norm_c(p, ch, saus, bcs):
            """Stage C (+3 iters): apply reciprocal, write an_sb."""
            for hf in range(2):
                nc.vector.tensor_mul(
                    an_sb[p][hf * DEPTH:(hf + 1) * DEPTH,
                             ch * CW:(ch + 1) * CW],
                    saus[hf][0:DEPTH, :], bcs[hf])

        def run_pair(p, filler, pre=None):
            """Emit one head-pair's attention with lag-LAG attn matmuls.

            filler: deque of (deadline, earliest, op); ops drain into PE
            slack once `earliest` (pair, ic) has passed, and are force-
            drained when their deadline (pair, chunk) arrives.
            """
            ha, hb = 2 * p, 2 * p + 1
            pend = deque()   # (ch, c, ew2, pattn) awaiting attn emission
            pattn_box = {}
            late = {}        # ic -> [deferred closures]
            pre = pre or {}

            def emit_attn(ic):
                ch, c, ew2, pattn = pend.popleft()
                nc.tensor.matmul(pattn[0], lhsT=v_sb[c][:, ha, 0:65],
                                 rhs=ew2[:, 0:CW],
                                 start=(c == 0), stop=(c == KT - 1))
                nc.tensor.matmul(pattn[1], lhsT=v_sb[c][:, hb, 0:65],
                                 rhs=ew2[:, CW:2 * CW],
                                 start=(c == 0), stop=(c == KT - 1))
                if c == KT - 1:
                    saus, den_t = norm_a(p, ch, pattn)
                    del pattn_box[ch]
                    box = {}
                    def stage_b(box=box, den_t=den_t):
                        box["bcs"] = norm_b(p, ch, den_t)
                    def stage_c(box=box, saus=saus):
                        norm_c(p, ch, saus, box["bcs"])
                    late.setdefault(ic + 3, []).append(stage_b)
                    late.setdefault(ic + 8, []).append(stage_c)

            for ic in range(NCH * KT):
                ch, c = divmod(ic, KT)
                for op in late.pop(ic, ()):
                    op()
                if c == 0:
                    while filler and filler[0][0] <= (p, ch):
                        filler.popleft()[2]()
                    # prefetch next eb slab (ring of 20 tiles), quartered
                    np_, nch = (p, ch + 1) if ch + 1 < NCH else (p + 1, 0)
                    if np_ < NPAIR:
                        nxt = [eb_tile(np_, nch, cc) for cc in range(KT)]
                        state["eb"][(np_, nch)] = nxt
                        state["ebnch"] = nch
                if c % 4 == 0:
                    np_, nch = (p, ch + 1) if ch + 1 < NCH else (p + 1, 0)
                    if np_ < NPAIR:
                        dma_eb_slab(state["ebnch"], state["eb"][(np_, nch)],
                                    c, c + 4)
                    pattn_box[ch] = [
                        pap.tile([65, CW], F32, tag="pattn",
                                 name=f"pa{p}_{ch}_{hf}", bufs=2)
                        for hf in range(2)]
                eb_cur = state["eb"][(p, ch)]
                plt = plp.tile([P, 2 * CW], F32, tag="pl",
                               name=f"pl{p}_{ch}_{c}", bufs=2)
                nc.tensor.matmul(plt[:, 0:CW],
                                 lhsT=kT_sb[p][0:DEPTH, c * P:(c + 1) * P],
                                 rhs=qT_sb[p][0:DEPTH, ch * CW:(ch + 1) * CW],
                                 start=True, stop=True)
                nc.tensor.matmul(plt[:, CW:2 * CW],
                                 lhsT=kT_sb[p][DEPTH:2 * DEPTH,
                                               c * P:(c + 1) * P],
                                 rhs=qT_sb[p][DEPTH:2 * DEPTH,
                                              ch * CW:(ch + 1) * CW],
                                 start=True, stop=True)
                ew = epool.tile([P, 2 * CW], BF, tag="ew",
                                name=f"ew{p}_{ch}_{c}", bufs=4)
                nc.scalar.activation(ew, plt, EXP)
                ew2 = epool.tile([P, 2 * CW], BF, tag="ew2",
                                 name=f"ew2{p}_{ch}_{c}", bufs=LAG + 2)
                nc.vector.tensor_mul(ew2[:, 0:CW], ew[:, 0:CW], eb_cur[c])
                nc.vector.tensor_mul(ew2[:, CW:2 * CW], ew[:, CW:2 * CW],
                                     eb_cur[c])
                pend.append((ch, c, ew2, pattn_box[ch]))
                if ic % 2 == 1:
                    while len(pend) > LAG:
                        emit_attn(ic)
                    for op in list(pre.pop(ic - 1, ())) + list(pre.pop(ic, ())):
                        op()
                    ndrain = 18 if ic < KT else 5
                    while ndrain > 0 and filler and filler[0][1] <= (p, ic):
                        filler.popleft()[2]()
                        ndrain -= 1
                if ch == NCH - 1 and c == KT - 1:
                    del state["eb"][(p, ch)]
            while pend:
                emit_attn(NCH * KT)
            for ic2 in sorted(late):
                for op in late[ic2]:
                    op()

        # ---- prologue compute: minimal critical path to first logits ----
        for op in k_group_ops(0, 0):
            op()
        for op in q_group_ops(0, 0):
            op()
        for c in range(2):
            for op in v_group_ops(c):
                op()

        # chunk-0 of pair 0 is DMA-paced: emit each v tile / kT chunk at the
        # iteration just before its first consumer so nothing blocks early
        pre0 = {}
        for c in range(2, KT):
            pre0.setdefault(c - 2, []).extend(v_group_ops(c))
        for kkc in (1, 2, 3):
            pre0.setdefault(4 * kkc - 1, []).extend(k_group_ops(0, kkc))

        fill = deque()
        ANY = (-1, -1)

        def add(dl, ops, earliest=ANY):
            fill.extend((dl, earliest, op) for op in ops)

        add((0, 1), q_group_ops(0, 1))
        for kkc in range(NCH):
            add((1, 0), k_group_ops(1, kkc))
        add((0, 2), q_group_ops(0, 2))
        add((1, 0), q_group_ops(1, 0))
        add((0, 3), q_group_ops(0, 3))
        run_pair(0, fill, pre0)

        add((1, 1), q_group_ops(1, 1))
        for kkc in range(NCH):
            add((2, 0), k_group_ops(2, kkc))
        add((1, 2), q_group_ops(1, 2))
        add((2, 0), q_group_ops(2, 0))
        add((1, 3), q_group_ops(1, 3))
        run_pair(1, fill)

        add((2, 1), q_group_ops(2, 1))
        for kkc in range(NCH):
            add((3, 0), k_group_ops(3, kkc))
        add((2, 2), q_group_ops(2, 2))
        add((3, 0), q_group_ops(3, 0))
        add((2, 3), q_group_ops(2, 3))
        add((3, 0), [lambda p=p: load_wo(p) for p in range(NPAIR)])
        add((3, 1), q_group_ops(3, 1))
        run_pair(2, fill)

        add((3, 2), q_group_ops(3, 2))
        add((3, 3), q_group_ops(3, 3))
        for ch in range(NCH - 1):
            for m in range(NU):
                add((3, ch + 2) if ch + 2 < NCH else (3, 3),
                    out_group_ops(m, ch),
                    earliest=(3, (ch + 1) * KT + 10))
        run_pair(3, fill)

        # tail: drain leftovers + last chunk's out-projection
        while fill:
            fill.popleft()[2]()
        for m in range(NU):
            for op in out_group_ops(m, NCH - 1):
                op()


def build_nc():
    nc = bacc.Bacc("TRN2", target_bir_lowering=False, debug=False)
    io = {
        "xT": nc.dram_tensor("xT", [NCH * D, CW], BF,
                             kind="ExternalInput").ap(),
        "yT": nc.dram_tensor("yT", [NCH * D, CW], BF,
                             kind="ExternalInput").ap(),
        "wqT": nc.dram_tensor("wqT", [NPAIR * D, P], BF,
                              kind="ExternalInput").ap(),
        "wkT": nc.dram_tensor("wkT", [NPAIR * D, P], BF,
                              kind="ExternalInput").ap(),
        "wvT": nc.dram_tensor("wvT", [D, DH], BF, kind="ExternalInput").ap(),
        "woT": nc.dram_tensor("woT", [DH, D], BF, kind="ExternalInput").ap(),
        "ebt": nc.dram_tensor("ebt", [NCH * S, CW], BF,
                              kind="ExternalInput").ap(),
        "outT": nc.dram_tensor("outT", [D, S], F32,
                               kind="ExternalOutput").ap(),
    }
    with tile.TileContext(nc) as tc:
        with ExitStack() as ctx:
            _attn_body(ctx, tc, io)
    nc.compile()
    return nc


_NC_CACHE = None


def kernel(x, y, bias, Wq, Wk, Wv, Wo):
    global _NC_CACHE, last_exec_time_ns, last_results
    x = np.asarray(x, np.float32)
    y = np.asarray(y, np.float32)
    bias = np.asarray(bias, np.float32)
    Wq, Wk, Wv, Wo = (np.asarray(w, np.float32) for w in (Wq, Wk, Wv, Wo))
    if _NC_CACHE is None:
        _NC_CACHE = build_nc()
    nc = _NC_CACHE

    bf = ml_dtypes.bfloat16
    scale = DEPTH ** -0.5
    wqT = np.ascontiguousarray(Wq.T * scale).astype(bf)
    wkT = np.ascontiguousarray(Wk.T).astype(bf)
    wvT = np.ascontiguousarray(Wv.T).astype(bf)
    woT = np.ascontiguousarray(Wo.T).astype(bf)

    # exp(bias).T pre-tiled: row ch*S + c*128 + p  <-  ebT[c*128+p, ch*512:+512]
    ebT = np.exp(bias[0, 0].astype(np.float32)).T
    ebt = np.ascontiguousarray(
        ebT.reshape(S, NCH, CW).transpose(1, 0, 2).reshape(NCH * S, CW)
    ).astype(bf)

    def chunk_slab(a):
        # [D, S] -> [NCH*D, CW]: row ch*D + r  <-  a[r, ch*CW:(ch+1)*CW]
        return np.ascontiguousarray(
            a.reshape(D, NCH, CW).transpose(1, 0, 2).reshape(NCH * D, CW))

    def pair_slab(a):
        # [D, DH] -> [NPAIR*D, P]: row p*D + r  <-  a[r, p*P:(p+1)*P]
        return np.ascontiguousarray(
            a.reshape(D, NPAIR, P).transpose(1, 0, 2).reshape(NPAIR * D, P))

    yT_all = [chunk_slab(y[b].T.astype(bf)) for b in range(B)]
    xT_all = [chunk_slab(x[b].T.astype(bf)) for b in range(B)]
    whalf = []
    for h in range(2):
        sl = slice(h * DH, (h + 1) * DH)
        whalf.append({
            "wqT": pair_slab(np.ascontiguousarray(wqT[:, sl])),
            "wkT": pair_slab(np.ascontiguousarray(wkT[:, sl])),
            "wvT": np.ascontiguousarray(wvT[:, sl]),
            "woT": np.ascontiguousarray(woT[sl, :]),
        })

    in_maps = []
    for core in range(NCORES):
        b, half = divmod(core, 2)
        m = {"xT": xT_all[b], "yT": yT_all[b], "ebt": ebt}
        m.update(whalf[half])
        in_maps.append(m)

    res = run_bass_kernel_spmd(nc, in_maps, core_ids=list(range(NCORES)),
                               trace=TRACE)
    last_exec_time_ns = res.exec_time_ns
    last_results = res
    out = np.empty((B, S, D), np.float32)
    for b in range(B):
        acc = res.results[2 * b]["outT"] + res.results[2 * b + 1]["outT"]
        out[b] = acc.T
    return out


# revision 25
# speedup vs baseline: 1.0294x; 1.0294x over previous
"""Multi-head attention with bias, distributed over 8 trn2 NeuronCores.

Reference computation (per batch b):
    q = (x @ Wq.T) * depth**-0.5 ; k = y @ Wk.T ; v = y @ Wv.T     (per-head split)
    out = softmax(q @ k.T + bias) @ v @ Wo.T

Sharding: 8 cores = 4 batches x 2 head-halves (tensor parallel over heads).
Core c handles batch b = c//2 and heads (c%2)*8 .. +8.  Wq/Wk/Wv are
column-split, Wo row-split; the two partial outputs per batch are summed on
the host (no device collective).

Device-side layout (feature dim on partitions):
    qT/kT = W.T-projected activations [d_out=512, 2048]; v natural [kk, h, d].
    logitsT[kk, q] per head via row-tiled K=64 matmul pairs (2 heads share
    the 128-partition d-tile; tile_position rows 0-63 / 64-127 concurrent)
    expw = exp(logitsT) * exp(bias).T      (exp(bias) precomputed on host,
                                            streamed per (pair, q-chunk))
    attnT_h(+denom row) = [v_h | ones].T @ expw  (K=128, denom rides row 64)
    normalize via DVE reciprocal + DMA partition-broadcast from DRAM
    outT_partial = Wo_half.T-proj of normalized attnT (summed on host).

Scheduling: attn matmuls LAG two iterations behind their logits pair so the
in-order PE queue never blocks on the ACT->DVE chain; the normalization
epilogue is split into three stages deferred across following iterations so
its DMA round-trips never head-block the DVE queue; projection matmuls are
chopped into per-MM filler ops drained into PE slack with (deadline,
earliest) emission gates.  Inputs arrive as one large striped DMA per
tensor in critical-path order.
Host does: transposes, bf16 casts, exp(bias) pre-tiling, scale fold into Wq.
"""

import numpy as np
import ml_dtypes
from collections import deque
from contextlib import ExitStack

import concourse.bass as bass
import concourse.mybir as mybir
import concourse.tile as tile
from concourse import bacc
from concourse.bass_utils import run_bass_kernel_spmd

# full-problem dims (hardcoded per spec)
B, S, D, H = 4, 2048, 1024, 16
DEPTH = D // H            # 64
P = 128
NCORES = 8

DH = D // 2               # 512 head dims per core (8 heads)
NPAIR = 4                 # head pairs per core
NCH = 4                   # q chunks of 512
KT = S // P               # 16 kk tiles
NU = D // P               # 8 d_in tiles
CW = 512                  # q chunk width
LAG = 2                   # attn matmul lag (iterations) behind logits

BF = mybir.dt.bfloat16
F32 = mybir.dt.float32
EXP = mybir.ActivationFunctionType.Exp

TRACE = False
last_exec_time_ns = None
last_results = None


def _attn_body(ctx, tc, io):
    nc = tc.nc
    xT, yT, wqT, wkT, wvT, woT, ebt, outT = (
        io[k] for k in ("xT", "yT", "wqT", "wkT", "wvT", "woT", "ebt", "outT"))

    # ---------------- persistent pools ----------------
    qpool = ctx.enter_context(tc.tile_pool(name="qpool", bufs=NPAIR))
    kpool = ctx.enter_context(tc.tile_pool(name="kpool", bufs=NPAIR))
    vpool = ctx.enter_context(tc.tile_pool(name="vpool", bufs=KT))
    anpool = ctx.enter_context(tc.tile_pool(name="anpool", bufs=NPAIR))
    ebpool = ctx.enter_context(tc.tile_pool(name="ebpool", bufs=16))
    epool = ctx.enter_context(tc.tile_pool(name="epool", bufs=8))
    smpool = ctx.enter_context(tc.tile_pool(name="smpool", bufs=4))
    plp = ctx.enter_context(tc.tile_pool(name="plp", bufs=2, space="PSUM"))
    pap = ctx.enter_context(tc.tile_pool(name="pap", bufs=2, space="PSUM"))
    pop = ctx.enter_context(tc.tile_pool(name="pop", bufs=2, space="PSUM"))
    dpool = ctx.enter_context(tc.tile_pool(name="dpool", bufs=2, space="DRAM"))
    wopool = ctx.enter_context(tc.tile_pool(name="wopool", bufs=NPAIR))
    opool = ctx.enter_context(tc.tile_pool(name="opool", bufs=2))

    qT_sb = [qpool.tile([P, S], BF, tag="qT", name=f"qT{p}", bufs=NPAIR)
             for p in range(NPAIR)]
    kT_sb = [kpool.tile([P, S], BF, tag="kT", name=f"kT{p}", bufs=NPAIR)
             for p in range(NPAIR)]
    v_sb = [vpool.tile([P, 2 * NPAIR, 66], BF, tag="v66", name=f"v{c}",
                       bufs=KT) for c in range(KT)]
    an_sb = [anpool.tile([P, S], BF, tag="an", name=f"an{p}", bufs=NPAIR)
             for p in range(NPAIR)]
    wo_sb = [wopool.tile([P, D], BF, tag="wo", name=f"wo{p}", bufs=NPAIR)
             for p in range(NPAIR)]

    def eb_tile(p, ch, c):
        return ebpool.tile([P, CW], BF, tag="eb", name=f"eb{p}_{ch}_{c}",
                           bufs=16)

    def dma_eb_slab(ch, tiles, c0=0, c1=KT):
        base = ch * S
        for c in range(c0, c1):
            nc.gpsimd.dma_start(out=tiles[c],
                                in_=ebt[base + c * P:base + (c + 1) * P, :])

    # ---------------- input loads + projection helpers ----------------
    with tc.tile_pool(name="ypool", bufs=1) as ypool, \
         tc.tile_pool(name="xpool", bufs=1) as xpool, \
         tc.tile_pool(name="wkpool", bufs=1) as wkpool, \
         tc.tile_pool(name="wqpool", bufs=1) as wqpool, \
         tc.tile_pool(name="wvpool", bufs=1) as wvpool:
        wk_sb = [wkpool.tile([P, NU, P], BF, tag="wk", name=f"wk{p}",
                             bufs=NPAIR) for p in range(NPAIR)]
        wq_sb = [wqpool.tile([P, NU, P], BF, tag="wq", name=f"wq{p}",
                             bufs=NPAIR) for p in range(NPAIR)]
        wv_sb = wvpool.tile([P, NU, DH], BF, tag="wv", name="wv")
        y_sb = ypool.tile([P, NU, S], BF, tag="yT", name="y")
        x_sb = xpool.tile([P, NU, S], BF, tag="xT", name="x")

        def dma_pair_w(dst, src, p):
            nc.sync.dma_start(
                out=dst[p],
                in_=src[p * D:(p + 1) * D, :].rearrange("(u p) j -> p u j",
                                                        p=P))

        def dma_slab(dst, src, ch):
            # 256KB units of 2 row-blocks: near-sequential DRAM read order
            for u in range(0, NU, 2):
                nc.sync.dma_start(
                    out=dst[:, u:u + 2, ch * CW:(ch + 1) * CW],
                    in_=src[ch * D + u * P:ch * D + (u + 2) * P, :].rearrange(
                        "(u p) j -> p u j", p=P))

        # pre-tiled transfers, strictly in first-use order
        dma_pair_w(wk_sb, wkT, 0)
        dma_slab(y_sb, yT, 0)
        dma_pair_w(wq_sb, wqT, 0)
        dma_slab(x_sb, xT, 0)
        for u in range(0, NU, 2):
            nc.sync.dma_start(
                out=wv_sb[:, u:u + 2, :],
                in_=wvT[u * P:(u + 2) * P, :].rearrange("(u p) j -> p u j",
                                                        p=P))
        dma_slab(y_sb, yT, 1)
        dma_slab(y_sb, yT, 2)
        dma_pair_w(wk_sb, wkT, 1)
        dma_slab(y_sb, yT, 3)
        dma_slab(x_sb, xT, 1)
        dma_pair_w(wq_sb, wqT, 1)
        dma_pair_w(wk_sb, wkT, 2)
        dma_pair_w(wk_sb, wkT, 3)
        dma_pair_w(wq_sb, wqT, 2)
        dma_pair_w(wq_sb, wqT, 3)
        dma_slab(x_sb, xT, 2)
        dma_slab(x_sb, xT, 3)
        # eb slab 0 on the gpsimd queue runs concurrently from t=0
        eb0 = [eb_tile(0, 0, c) for c in range(KT)]
        dma_eb_slab(0, eb0)

        # warm-up heartbeats chained to arriving inputs keep the PE HAM
        # activity window alive through the load phase
        jnk0 = plp.tile([1, 1024], F32, tag="pl", name="jnk0", bufs=2)
        for t in (wk_sb[0], wv_sb, y_sb):
            nc.tensor.matmul(jnk0[0:1, 0:P], lhsT=t[0:1, 0, 0:1],
                             rhs=t[0:1, 0, 0:P], start=True, stop=True)

        # ---- per-MM filler ops ----
        def k_group_ops(p, kkc):
            box = {}
            def mm(u, box=box):
                if u == 0:
                    box["ps"] = pop.tile([P, CW], F32, tag="po",
                                         name=f"psk{p}_{kkc}", bufs=2)
                nc.tensor.matmul(box["ps"],
                                 lhsT=wk_sb[p][:, u, :],
                                 rhs=y_sb[:, u, kkc * CW:(kkc + 1) * CW],
                                 start=(u == 0), stop=(u == NU - 1))
            def fin(box=box):
                nc.vector.tensor_copy(
                    kT_sb[p][:, kkc * CW:(kkc + 1) * CW], box["ps"])
            return [lambda u=u, mm=mm: mm(u) for u in range(NU)] + [fin]

        def q_group_ops(p, ch):
            box = {}
            def mm(u, box=box):
                if u == 0:
                    box["ps"] = pop.tile([P, CW], F32, tag="po",
                                         name=f"psq{p}_{ch}", bufs=2)
                nc.tensor.matmul(box["ps"],
                                 lhsT=wq_sb[p][:, u, :],
                                 rhs=x_sb[:, u, ch * CW:(ch + 1) * CW],
                                 start=(u == 0), stop=(u == NU - 1))
            def fin(box=box):
                nc.vector.tensor_copy(
                    qT_sb[p][:, ch * CW:(ch + 1) * CW], box["ps"])
            return [lambda u=u, mm=mm: mm(u) for u in range(NU)] + [fin]

        def v_group_ops(c):
            box = {}
            def mm(u, box=box):
                if u == 0:
                    box["ps"] = pop.tile([P, CW], F32, tag="po",
                                         name=f"psv{c}", bufs=2)
                nc.tensor.matmul(box["ps"],
                                 lhsT=y_sb[:, u, c * P:(c + 1) * P],
                                 rhs=wv_sb[:, u, 0:DH],
                                 start=(u == 0), stop=(u == NU - 1))
            def fin(box=box):
                vt = v_sb[c]
                nc.vector.memset(vt[:, :, 64:65], 1.0)
                nc.vector.memset(vt[:, :, 65:66], 0.0)
                src = box["ps"].rearrange("p (g d) -> p g d", d=DEPTH)
                nc.vector.tensor_copy(vt[:, :, 0:DEPTH], src)
            return [lambda u=u, mm=mm: mm(u) for u in range(NU)] + [fin]

        def load_wo(p):
            nc.gpsimd.dma_start(out=wo_sb[p], in_=woT[p * P:(p + 1) * P, :])

        def out_group_ops(m, ch):
            box = {}
            def mm(p4, box=box):
                if p4 == 0:
                    box["ps"] = pop.tile([P, CW], F32, tag="po",
                                         name=f"pso{m}_{ch}", bufs=2)
                nc.tensor.matmul(box["ps"],
                                 lhsT=wo_sb[p4][:, m * P:(m + 1) * P],
                                 rhs=an_sb[p4][:, ch * CW:(ch + 1) * CW],
                                 start=(p4 == 0), stop=(p4 == NPAIR - 1))
            def fin(box=box):
                osb = opool.tile([P, CW], BF, tag="osb", name=f"o{m}_{ch}",
                                 bufs=2)
                nc.vector.tensor_copy(osb, box["ps"])
                nc.sync.dma_start(
                    out=outT[m * P:(m + 1) * P, ch * CW:(ch + 1) * CW],
                    in_=osb)
            return [lambda p4=p4, mm=mm: mm(p4) for p4 in range(NPAIR)] + [fin]

        # ---------------- attention emission ----------------
        state = {"eb": {(0, 0): eb0}}

        def norm_a(p, ch, pattn):
            """Stage A (at last attn MM): drain psum, gather denominators."""
            saus = []
            for hf in range(2):
                sau = smpool.tile([65, CW], BF, tag="sau",
                                  name=f"sa{p}_{ch}_{hf}", bufs=3)
                saus.append(sau)
                nc.vector.tensor_copy(sau, pattn[hf])
            den_t = smpool.tile([2, CW], BF, tag="dent",
                                name=f"den{p}_{ch}", bufs=1)
            for hf in range(2):
                nc.sync.dma_start(out=den_t[hf:hf + 1, :],
                                  in_=saus[hf][64:65, :])
            return saus, den_t

        def norm_b(p, ch, den_t):
            """Stage B (+1 iter): reciprocal + DRAM broadcast round-trip."""
            denf = smpool.tile([2, CW], F32, tag="denf",
                               name=f"dnf{p}_{ch}", bufs=1)
            nc.vector.tensor_copy(denf, den_t)
            recipf = smpool.tile([2, CW], F32, tag="recipf",
                                 name=f"rcf{p}_{ch}", bufs=1)
            nc.vector.reciprocal_approx_fast(recipf, denf)
            recipb = smpool.tile([2, CW], BF, tag="recipb",
                                 name=f"rcb{p}_{ch}", bufs=1)
            nc.vector.tensor_copy(recipb, recipf)
            rscr = dpool.tile([2, CW], BF, tag="rscr",
                              name=f"rs{p}_{ch}", bufs=2)
            nc.sync.dma_start(out=rscr, in_=recipb)
            bcs = []
            for hf in range(2):
                bc = smpool.tile([DEPTH, CW], BF, tag="bc",
                                 name=f"bc{p}_{ch}_{hf}", bufs=2)
                bcs.append(bc)
                nc.sync.dma_start(
                    out=bc, in_=rscr[hf:hf + 1, :].partition_broadcast(DEPTH))
            return bcs

        def norm_c(p, ch, saus, bcs):
            """Stage C (+3 iters): apply reciprocal, write an_sb."""
            for hf in range(2):
                nc.vector.tensor_mul(
                    an_sb[p][hf * DEPTH:(hf + 1) * DEPTH,
                             ch * CW:(ch + 1) * CW],
                    saus[hf][0:DEPTH, :], bcs[hf])

        def run_pair(p, filler, pre=None):
            """Emit one head-pair's attention with lag-LAG attn matmuls.

            filler: deque of (deadline, earliest, op); ops drain into PE
            slack once `earliest` (pair, ic) has passed, and are force-
            drained when their deadline (pair, chunk) arrives.
            """
            ha, hb = 2 * p, 2 * p + 1
            pend = deque()   # (ch, c, ew2, pattn) awaiting attn emission
            pattn_box = {}
            late = {}        # ic -> [deferred closures]
            pre = pre or {}

            def emit_attn(ic):
                ch, c, ew2, pattn = pend.popleft()
                nc.tensor.matmul(pattn[0], lhsT=v_sb[c][:, ha, 0:65],
                                 rhs=ew2[:, 0:CW],
                                 start=(c == 0), stop=(c == KT - 1))
                nc.tensor.matmul(pattn[1], lhsT=v_sb[c][:, hb, 0:65],
                                 rhs=ew2[:, CW:2 * CW],
                                 start=(c == 0), stop=(c == KT - 1))
                if c == KT - 1:
                    saus, den_t = norm_a(p, ch, pattn)
                    del pattn_box[ch]
                    box = {}
                    def stage_b(box=box, den_t=den_t):
                        box["bcs"] = norm_b(p, ch, den_t)
                    def stage_c(box=box, saus=saus):
                        norm_c(p, ch, saus, box["bcs"])
                    late.setdefault(ic + 3, []).append(stage_b)
                    late.setdefault(ic + 8, []).append(stage_c)

            for ic in range(NCH * KT):
                ch, c = divmod(ic, KT)
                for op in late.pop(ic, ()):
                    op()
                if c == 0:
                    while filler and filler[0][0] <= (p, ch):
                        filler.popleft()[2]()
                    # prefetch next eb slab (ring of 20 tiles), quartered
                    np_, nch = (p, ch + 1) if ch + 1 < NCH else (p + 1, 0)
                    if np_ < NPAIR:
                        nxt = [eb_tile(np_, nch, cc) for cc in range(KT)]
                        state["eb"][(np_, nch)] = nxt
                        state["ebnch"] = nch
                if c % 4 == 0:
                    np_, nch = (p, ch + 1) if ch + 1 < NCH else (p + 1, 0)
                    if np_ < NPAIR:
                        dma_eb_slab(state["ebnch"], state["eb"][(np_, nch)],
                                    c, c + 4)
                    pattn_box[ch] = [
                        pap.tile([65, CW], F32, tag="pattn",
                                 name=f"pa{p}_{ch}_{hf}", bufs=2)
                        for hf in range(2)]
                eb_cur = state["eb"][(p, ch)]
                plt = plp.tile([P, 2 * CW], F32, tag="pl",
                               name=f"pl{p}_{ch}_{c}", bufs=2)
                nc.tensor.matmul(plt[:, 0:CW],
                                 lhsT=kT_sb[p][0:DEPTH, c * P:(c + 1) * P],
                                 rhs=qT_sb[p][0:DEPTH, ch * CW:(ch + 1) * CW],
                                 start=True, stop=True)
                nc.tensor.matmul(plt[:, CW:2 * CW],
                                 lhsT=kT_sb[p][DEPTH:2 * DEPTH,
                                               c * P:(c + 1) * P],
                                 rhs=qT_sb[p][DEPTH:2 * DEPTH,
                                              ch * CW:(ch + 1) * CW],
                                 start=True, stop=True)
                ew = epool.tile([P, 2 * CW], BF, tag="ew",
                                name=f"ew{p}_{ch}_{c}", bufs=4)
                nc.scalar.activation(ew, plt, EXP)
                ew2 = epool.tile([P, 2 * CW], BF, tag="ew2",
                                 name=f"ew2{p}_{ch}_{c}", bufs=LAG + 2)
                nc.vector.tensor_mul(ew2[:, 0:CW], ew[:, 0:CW], eb_cur[c])
                nc.vector.tensor_mul(ew2[:, CW:2 * CW], ew[:, CW:2 * CW],
                                     eb_cur[c])
                pend.append((ch, c, ew2, pattn_box[ch]))
                if ic % 2 == 1:
                    while len(pend) > LAG:
                        emit_attn(ic)
                    for op in list(pre.pop(ic - 1, ())) + list(pre.pop(ic, ())):
                        op()
                    ndrain = 18 if ic < KT else 5
                    while ndrain > 0 and filler and filler[0][1] <= (p, ic):
                        filler.popleft()[2]()
                        ndrain -= 1
                if ch == NCH - 1 and c == KT - 1:
                    del state["eb"][(p, ch)]
            while pend:
                emit_attn(NCH * KT)
            for ic2 in sorted(late):
                for op in late[ic2]:
                    op()

        # ---- prologue compute: minimal critical path to first logits ----
        for op in k_group_ops(0, 0):
            op()
        for op in q_group_ops(0, 0):
            op()
        for c in range(2):
            for op in v_group_ops(c):
                op()

        # chunk-0 of pair 0 is DMA-paced: emit each v tile / kT chunk at the
        # iteration just before its first consumer so nothing blocks early
        pre0 = {}
        for c in range(2, KT):
            pre0.setdefault(c - 2, []).extend(v_group_ops(c))
        for kkc in (1, 2, 3):
            pre0.setdefault(4 * kkc - 1, []).extend(k_group_ops(0, kkc))

        fill = deque()
        ANY = (-1, -1)

        def add(dl, ops, earliest=ANY):
            fill.extend((dl, earliest, op) for op in ops)

        add((0, 1), q_group_ops(0, 1))
        for kkc in range(NCH):
            add((1, 0), k_group_ops(1, kkc))
        add((0, 2), q_group_ops(0, 2))
        add((1, 0), q_group_ops(1, 0))
        add((0, 3), q_group_ops(0, 3))
        run_pair(0, fill, pre0)

        add((1, 1), q_group_ops(1, 1))
        for kkc in range(NCH):
            add((2, 0), k_group_ops(2, kkc))
        add((1, 2), q_group_ops(1, 2))
        add((2, 0), q_group_ops(2, 0))
        add((1, 3), q_group_ops(1, 3))
        run_pair(1, fill)

        add((2, 1), q_group_ops(2, 1))
        for kkc in range(NCH):
            add((3, 0), k_group_ops(3, kkc))
        add((2, 2), q_group_ops(2, 2))
        add((3, 0), q_group_ops(3, 0))
        add((2, 3), q_group_ops(2, 3))
        add((3, 0), [lambda p=p: load_wo(p) for p in range(NPAIR)])
        add((3, 1), q_group_ops(3, 1))
        run_pair(2, fill)

        add((3, 2), q_group_ops(3, 2))
        add((3, 3), q_group_ops(3, 3))
        for ch in range(NCH - 1):
            for m in range(NU):
                add((3, ch + 2) if ch + 2 < NCH else (3, 3),
                    out_group_ops(m, ch),
                    earliest=(3, (ch + 1) * KT + 10))
        run_pair(3, fill)

        # tail: drain leftovers + last chunk's out-projection
        while fill:
            fill.popleft()[2]()
        for m in range(NU):
            for op in out_group_ops(m, NCH - 1):
                op()


def build_nc():
    nc = bacc.Bacc("TRN2", target_bir_lowering=False, debug=False)
    io = {
        "xT": nc.dram_tensor("xT", [NCH * D, CW], BF,
                             kind="ExternalInput").ap(),
        "yT": nc.dram_tensor("yT", [NCH * D, CW], BF,
                             kind="ExternalInput").ap(),
        "wqT": nc.dram_tensor("wqT", [NPAIR * D, P], BF,
                              kind="ExternalInput").ap(),
        "wkT": nc.dram_tensor("wkT", [NPAIR * D, P], BF,
                              kind="ExternalInput").ap(),
        "wvT": nc.dram_tensor("wvT", [D, DH], BF, kind="ExternalInput").ap(),
        "woT": nc.dram_tensor("woT", [DH, D], BF, kind="ExternalInput").ap(),
        "ebt": nc.dram_tensor("ebt", [NCH * S, CW], BF,
                              kind="ExternalInput").ap(),
        "outT": nc.dram_tensor("outT", [D, S], BF,
                               kind="ExternalOutput").ap(),
    }
    with tile.TileContext(nc) as tc:
        with ExitStack() as ctx:
            _attn_body(ctx, tc, io)
    nc.compile()
    return nc


_NC_CACHE = None


def kernel(x, y, bias, Wq, Wk, Wv, Wo):
    global _NC_CACHE, last_exec_time_ns, last_results
    x = np.asarray(x, np.float32)
    y = np.asarray(y, np.float32)
    bias = np.asarray(bias, np.float32)
    Wq, Wk, Wv, Wo = (np.asarray(w, np.float32) for w in (Wq, Wk, Wv, Wo))
    if _NC_CACHE is None:
        _NC_CACHE = build_nc()
    nc = _NC_CACHE

    bf = ml_dtypes.bfloat16
    scale = DEPTH ** -0.5
    wqT = np.ascontiguousarray(Wq.T * scale).astype(bf)
    wkT = np.ascontiguousarray(Wk.T).astype(bf)
    wvT = np.ascontiguousarray(Wv.T).astype(bf)
    woT = np.ascontiguousarray(Wo.T).astype(bf)

    # exp(bias).T pre-tiled: row ch*S + c*128 + p  <-  ebT[c*128+p, ch*512:+512]
    ebT = np.exp(bias[0, 0].astype(np.float32)).T
    ebt = np.ascontiguousarray(
        ebT.reshape(S, NCH, CW).transpose(1, 0, 2).reshape(NCH * S, CW)
    ).astype(bf)

    def chunk_slab(a):
        # [D, S] -> [NCH*D, CW]: row ch*D + r  <-  a[r, ch*CW:(ch+1)*CW]
        return np.ascontiguousarray(
            a.reshape(D, NCH, CW).transpose(1, 0, 2).reshape(NCH * D, CW))

    def pair_slab(a):
        # [D, DH] -> [NPAIR*D, P]: row p*D + r  <-  a[r, p*P:(p+1)*P]
        return np.ascontiguousarray(
            a.reshape(D, NPAIR, P).transpose(1, 0, 2).reshape(NPAIR * D, P))

    yT_all = [chunk_slab(y[b].T.astype(bf)) for b in range(B)]
    xT_all = [chunk_slab(x[b].T.astype(bf)) for b in range(B)]
    whalf = []
    for h in range(2):
        sl = slice(h * DH, (h + 1) * DH)
        whalf.append({
            "wqT": pair_slab(np.ascontiguousarray(wqT[:, sl])),
            "wkT": pair_slab(np.ascontiguousarray(wkT[:, sl])),
            "wvT": np.ascontiguousarray(wvT[:, sl]),
            "woT": np.ascontiguousarray(woT[sl, :]),
        })

    in_maps = []
    for core in range(NCORES):
        b, half = divmod(core, 2)
        m = {"xT": xT_all[b], "yT": yT_all[b], "ebt": ebt}
        m.update(whalf[half])
        in_maps.append(m)

    res = run_bass_kernel_spmd(nc, in_maps, core_ids=list(range(NCORES)),
                               trace=TRACE)
    last_exec_time_ns = res.exec_time_ns
    last_results = res
    out = np.empty((B, S, D), np.float32)
    for b in range(B):
        acc = (res.results[2 * b]["outT"].astype(np.float32) +
               res.results[2 * b + 1]["outT"].astype(np.float32))
        out[b] = acc.T
    return out
